# revision 12
# baseline (speedup 1.0000x reference)
"""LSQ weight quantization (4-bit, uniform power-of-2 level grid) on 8 NeuronCores.

Reference semantics: per output-channel c, build 81 quantization levels
sum_i(scales_sc[i,c] * e_i), e in {-1,0,1}^4 with scales_sc[i] = base/2^i;
pick nearest level per element (argmin over fp32 |x - level|, first-index
tie-break).  Because the 4 scale rows are exact powers of two of each other,
the 81 levels collapse to a uniform grid {k * u : k = -15..15}, u =
scales_sc[3].  So the per-element work is round(x/u), clip to [-15,15],
multiply back by u — a pure memory-bound elementwise pass.

Device kernel (per core, 64 of 512 channels, laid out 128 partitions x 2304;
per-partition scalars r=fl(1/u) and u ride as 2 extra columns of chunk 0 so
every compute instruction needs at most ONE sync wait — this ISA build allows
a single sync-wait slot per compute instruction):
  ACT: t = x * r_c          (Identity activation, per-partition scale)
  DVE: a = max(t + MAGIC, MAGIC-15)       # MAGIC=1.5*2^23 rounds half-even
  DVE: b = min(a, MAGIC+15) - MAGIC
  ACT: out = b * u_c

Host patches the measure-zero set of elements whose x/u lands within 1e-4 of
a rounding boundary, using the exact 81-level fp32 argmin (the reference's
fl-computed level table differs from k*u by ~1 ulp, which can flip the
nearest-level decision for elements essentially ON a boundary).
"""

import numpy as np

import concourse.bass as bass
import concourse.tile as tile
from concourse import mybir
from concourse.bass_utils import run_bass_kernel_spmd

BIT = 4
C, CIN, KH, KW = 512, 512, 3, 3
N = CIN * KH * KW              # 4608 elements per channel
NCORES = 8
CPC = C // NCORES              # 64 channels per core
P = 128                        # SBUF partitions; 2 partitions per channel
W = CPC * N // P               # 2304 columns per partition
CHUNKS = 4
CW = W // CHUNKS               # 576 columns per chunk
MAGIC = float(np.float32(12582912.0))   # 1.5 * 2^23
F32 = mybir.dt.float32

_prog_cache = {}


def _build_program():
    if "nc" in _prog_cache:
        return _prog_cache["nc"]
    nc = bass.Bass()
    # input: [r | u | x chunk 0 | x chunk 1 | ...] so chunk 0's DMA also
    # delivers the per-partition scalars (single-dependency rule).
    x_ext = nc.declare_dram_parameter("xr", [P, W + 2], F32, isOutput=False)
    out_ext = nc.declare_dram_parameter("out", [P, W], F32, isOutput=True)

    op = mybir.AluOpType
    ident = mybir.ActivationFunctionType.Identity
    with tile.TileContext(nc) as tc:
        with (
            tc.tile_pool(name="c0", bufs=1) as c0pool,
            tc.tile_pool(name="io", bufs=4) as iopool,
            tc.tile_pool(name="tmp", bufs=4) as tpool,
        ):
            xt0 = c0pool.tile([P, 2 + CW], F32)
            nc.sync.dma_start(out=xt0, in_=x_ext[:, 0: 2 + CW])
            r_ap = xt0[:, 0:1]
            u_ap = xt0[:, 1:2]
            for j in range(CHUNKS):
                if j == 0:
                    xin = xt0[:, 2: 2 + CW]
                else:
                    xt = iopool.tile([P, CW], F32, tag="xt")
                    nc.sync.dma_start(out=xt, in_=x_ext[:, 2 + j * CW: 2 + (j + 1) * CW])
                    xin = xt[:, :]
                a = tpool.tile([P, CW], F32, tag="a")
                nc.vector.tensor_scalar(a, xin, r_ap, MAGIC, op.mult, op.add)
                b = tpool.tile([P, CW], F32, tag="b")
                nc.vector.tensor_scalar(b, a, MAGIC - 15.0, MAGIC + 15.0, op.max, op.min)
                o = iopool.tile([P, CW], F32, tag="ot")
                nc.vector.tensor_scalar(o, b, MAGIC, u_ap, op.subtract, op.mult)
                nc.sync.dma_start(out=out_ext[:, j * CW: (j + 1) * CW], in_=o)
    _split_drain_waits(nc)
    _prog_cache["nc"] = nc
    return nc


def _split_drain_waits(nc):
    # This walrus build allows very few sync-wait slots per instruction; the
    # Tile tail drain carries one wait per semaphore lane used.  Split it into
    # a chain of single-wait drains (semantically identical: each drains and
    # waits; the last one keeps the update).
    import copy as _copy

    for f in nc.m.functions:
        for blk in f.blocks:
            for pos, inst in enumerate(list(blk.instructions)):
                si = getattr(inst, "sync_info", None)
                if (
                    type(inst).__name__ == "InstDrain"
                    and si is not None
                    and si.on_wait
                    and len(si.on_wait) > 1
                ):
                    waits = list(si.on_wait)
                    clones = []
                    for idx, w in enumerate(waits[:-1]):
                        cl = _copy.deepcopy(inst)
                        cl.name = f"{inst.name}_ws{idx}"
                        cl.sync_info.on_wait = [w]
                        cl.sync_info.on_update = []
                        clones.append(cl)
                    si.on_wait = [waits[-1]]
                    real_pos = blk.instructions.index(inst)
                    blk.instructions[real_pos:real_pos] = clones


def _scales_sc(scales):
    # fp32-faithful replication of: stop_grad(scales*(1-g)) + scales*g
    g = np.float32(1.0 / float(np.sqrt((2 ** BIT - 1) * N)) / 2.0 ** (BIT - 2))
    one = np.float32(1.0)
    return (scales * (one - g) + scales * g).astype(np.float32)


def _levels_table(s):
    # fp32-faithful replication of jnp.einsum('ic,li->cl', s, coeff):
    # sequential accumulation over i for each (c, l).
    vals = np.array([-1.0, 0.0, 1.0], dtype=np.float32)
    grids = np.meshgrid(*([vals] * BIT), indexing="ij")
    coeff = np.stack([gr.reshape(-1) for gr in grids], axis=1)  # [81, 4]
    acc = np.zeros((s.shape[1], coeff.shape[0]), dtype=np.float32)  # [C, 81]
    for i in range(BIT):
        acc = (acc + s[i][:, None] * coeff[None, :, i]).astype(np.float32)
    return acc


def _make_in_maps(x, scales):
    s = _scales_sc(scales)
    u = s[BIT - 1]                                     # [C] grid spacing
    r = (np.float32(1.0) / u).astype(np.float32)       # [C]
    xf = x.reshape(C, N)
    in_maps = []
    for core in range(NCORES):
        ch = slice(core * CPC, (core + 1) * CPC)
        xs = xf[ch].reshape(P, W)
        ru = np.stack(
            [np.repeat(r[ch], 2), np.repeat(u[ch], 2)], axis=1
        ).astype(np.float32)
        xr = np.ascontiguousarray(np.concatenate([ru, xs], axis=1))
        in_maps.append({"xr": xr})
    return in_maps, s, u, r, xf


def kernel(x, scales):
    x = np.ascontiguousarray(x, dtype=np.float32)
    scales = np.asarray(scales, dtype=np.float32)

    in_maps, s, u, r, xf = _make_in_maps(x, scales)
    nc = _build_program()
    res = run_bass_kernel_spmd(nc, in_maps, list(range(NCORES))).results

    out = np.empty((C, N), dtype=np.float32)
    for core in range(NCORES):
        out[core * CPC: (core + 1) * CPC] = res[core]["out"].reshape(CPC, N)

    # Exact-reference patch for elements within 1e-4 of a decision boundary.
    t = (xf * r[:, None]).astype(np.float32)
    frac = t - np.floor(t)
    band = (np.abs(frac - np.float32(0.5)) < 1e-4) & (np.abs(t) < 15.5)
    if band.any():
        lv = _levels_table(s)                          # [C, 81]
        cc, nn = np.nonzero(band)
        d = np.abs(xf[cc, nn][:, None] - lv[cc]).astype(np.float32)
        l = np.argmin(d, axis=1)
        out[cc, nn] = lv[cc, l]

    return out.reshape(x.shape)
